# revision 10
# baseline (speedup 1.0000x reference)
"""Trainium2 Bass kernel for nn_AbstractRelu (DeepPoly abstract-ReLU transform).

The reference's piecewise-linear transform reduces exactly to:
    x_out    = relu(x)
    high_out = relu(high)        (crossing branch: w_high*high + b_high == high)
    low_out  = low if low + high >= 0 else 0
and `relu(high)` can replace `high` in the low_out test without changing any
result (when high <= 0, low < high <= 0 forces low + high < 0 AND low < 0).

Sharding: N=16.7M elements split evenly across 8 NeuronCores; fully
elementwise, no communication.
"""

import numpy as np

import concourse.bass as bass
import concourse.bacc as bacc
import concourse.mybir as mybir
from concourse.tile import TileContext
from concourse.bass_utils import run_bass_kernel_spmd

N = 16777216
N_CORES = 8
SHARD = N // N_CORES  # 2_097_152
P = 128
FREE = SHARD // P  # 16384 f32 per partition per core (64 KiB)
TILE_COLS = 4096  # 2 MiB per DMA transfer
F32 = mybir.dt.float32


def build_program(
    free: int = FREE,
    tile_cols: int = TILE_COLS,
    bufs: int = 2,
    repeats: int = 1,
    hw_loop_repeats: int = 1,
    inplace_low: bool = False,
) -> bass.Bass:
    """hw_loop_repeats wraps the whole body in a tc.For_i hardware loop —
    used only by the timing harness (repeat-differencing).
    inplace_low computes low_out inside the high tile (3 SBUF tags instead
    of 4, allowing larger tiles)."""
    assert free % tile_cols == 0
    n_tiles = free // tile_cols

    nc = bacc.Bacc(
        "TRN2", target_bir_lowering=False, debug=False, num_devices=N_CORES
    )
    x = nc.declare_dram_parameter("x", [P, free], F32, isOutput=False)
    low = nc.declare_dram_parameter("low", [P, free], F32, isOutput=False)
    high = nc.declare_dram_parameter("high", [P, free], F32, isOutput=False)
    x_out = nc.declare_dram_parameter("x_out", [P, free], F32, isOutput=True)
    low_out = nc.declare_dram_parameter("low_out", [P, free], F32, isOutput=True)
    high_out = nc.declare_dram_parameter("high_out", [P, free], F32, isOutput=True)

    relu = mybir.ActivationFunctionType.Relu
    with TileContext(nc) as tc:
        with tc.tile_pool(name="io", bufs=bufs) as pool:

            def body():
                for t in range(n_tiles * repeats):
                    sl = bass.ts(t % n_tiles, tile_cols)

                    xt = pool.tile([P, tile_cols], F32, tag="x")
                    nc.sync.dma_start(out=xt[:], in_=x[:, sl])
                    nc.scalar.activation(xt[:], xt[:], relu)
                    nc.scalar.dma_start(out=x_out[:, sl], in_=xt[:])

                    ht = pool.tile([P, tile_cols], F32, tag="h")
                    nc.sync.dma_start(out=ht[:], in_=high[:, sl])
                    lt = pool.tile([P, tile_cols], F32, tag="l")
                    nc.sync.dma_start(out=lt[:], in_=low[:, sl])

                    nc.scalar.activation(ht[:], ht[:], relu)
                    nc.scalar.dma_start(out=high_out[:, sl], in_=ht[:])

                    tt = ht if inplace_low else pool.tile([P, tile_cols], F32, tag="t")
                    nc.vector.tensor_add(tt[:], lt[:], ht[:])
                    nc.vector.tensor_scalar(
                        tt[:], tt[:], 0.0, None, mybir.AluOpType.is_ge
                    )
                    nc.vector.tensor_mul(tt[:], tt[:], lt[:])
                    nc.scalar.dma_start(out=low_out[:, sl], in_=tt[:])

            if hw_loop_repeats > 1:
                with tc.For_i(0, hw_loop_repeats, 1):
                    body()
            else:
                body()
    nc.compile()
    return nc


_NC = None


def _get_nc() -> bass.Bass:
    global _NC
    if _NC is None:
        _NC = build_program()
    return _NC


def kernel(x: np.ndarray, low: np.ndarray, high: np.ndarray, **_run_kwargs):
    nc = _get_nc()
    in_maps = []
    for c in range(N_CORES):
        s = slice(c * SHARD, (c + 1) * SHARD)
        in_maps.append(
            {
                "x": np.ascontiguousarray(x[s]).reshape(P, FREE),
                "low": np.ascontiguousarray(low[s]).reshape(P, FREE),
                "high": np.ascontiguousarray(high[s]).reshape(P, FREE),
            }
        )
    res = run_bass_kernel_spmd(nc, in_maps, list(range(N_CORES)), **_run_kwargs)
    results = res.results
    x_out = np.concatenate([results[c]["x_out"].reshape(-1) for c in range(N_CORES)])
    low_out = np.concatenate([results[c]["low_out"].reshape(-1) for c in range(N_CORES)])
    high_out = np.concatenate([results[c]["high_out"].reshape(-1) for c in range(N_CORES)])
    if _run_kwargs:
        kernel.last_results = res  # expose trace/profile to test harness
    return (
        x_out.astype(np.float32, copy=False),
        low_out.astype(np.float32, copy=False),
        high_out.astype(np.float32, copy=False),
    )
